# revision 19
# baseline (speedup 1.0000x reference)
"""Differential attention Trainium2 kernel (8 NeuronCores, SPMD).

Sharding: core c handles batch c//4, heads [4*(c%4), 4*(c%4)+4).

Math per core (one batch beta, 4 heads split into 2 pairs):
  qkv^T = (x @ w_qkv)^T computed as w^T-chunks x x^T-chunks (bf16 matmuls,
  fp32 PSUM accumulate)
  Per head pair, per map m in {1,2}:
    S^T[t, q] = K_m^T(.T) @ Q_m^T        (K=32 row-tiled 4-way packed, bf16)
    E^T = exp(S^T / 8)                    (ACT, PSUM->SBUF, bf16 out)
    O^T_aug[65, q] += V_aug[t,65].T @ E^T (bf16, ones-column gives row sums r)
  Epilogue (per pair, per 512-query chunk, fp32):
    transpose O^T_aug -> [q, 65]; neg_mu = -lam*r1/r2 (per-q scalar);
    pre = O1 + neg_mu*O2;  rsq = rsqrt(mean(pre^2)+eps) via bit-trick+Newton;
    On[t, d] = pre*rsq  (RMSNorm scale-invariance cancels 1/r1; norm_w*0.2
    folded into the final wo-phase evacuation).
  Final (replicates reference's cat(dim=1)->transpose->view quirk):
    F[d-block rows] = On_half[t,d].T @ wo  contracting over 1024-token halves.
    Output rows are disjoint across cores -> host scatter, no all-reduce.
"""
import numpy as np

B, S, E, H = 2, 2048, 1024, 16
HD = 64
LAMBDA_INIT = 0.8
EPS = 1e-6

_CACHE = {}
USE_TP = True  # tile_position row-packing for QK


def _build_nc():
    import concourse.bass as bass
    import concourse.tile as tile
    from concourse import bacc, mybir

    f32 = mybir.dt.float32
    bf = mybir.dt.bfloat16
    i32 = mybir.dt.int32
    FT = mybir.ActivationFunctionType
    OP = mybir.AluOpType

    nc = bacc.Bacc("TRN2", target_bir_lowering=False, debug=False, num_devices=8)

    xt_d = nc.dram_tensor("xt", [E, S], bf, kind="ExternalInput").ap()
    wq_d = nc.dram_tensor("wq", [128, 6 * 8 * 128], bf, kind="ExternalInput").ap()
    wo_d = nc.dram_tensor("wo_full", [E, E], bf, kind="ExternalInput").ap()
    nw_d = nc.dram_tensor("nw", [2, 128, 1], f32, kind="ExternalInput").ap()
    lamq_d = nc.dram_tensor("lamq", [HD, 256], f32, kind="ExternalInput").ap()
    lamk_d = nc.dram_tensor("lamk", [HD, 2], f32, kind="ExternalInput").ap()
    id_d = nc.dram_tensor("ident", [128, 128], f32, kind="ExternalInput").ap()
    out_d = nc.dram_tensor("outp", [4, 2, HD, E], f32, kind="ExternalOutput").ap()

    MAGIC = 0x5F3759DF

    with tile.TileContext(nc) as tc:
        with (
            tc.tile_pool(name="consts", bufs=1) as consts,
            tc.tile_pool(name="qkv", bufs=1) as qkv,
            tc.tile_pool(name="vbuf", bufs=1) as vbuf,
            tc.tile_pool(name="onb", bufs=1) as onb,
        ):
            # ---- constants ----
            wq_sb = consts.tile([128, 6 * 8 * 128], bf)
            nc.sync.dma_start(wq_sb[:], wq_d[:])
            wqv = wq_sb.rearrange("p (m k c) -> p m k c", m=6, k=8)
            wo_sb = consts.tile([128, 8 * E], bf)
            wov = wo_sb.rearrange("p (k j) -> p k j", k=8)
            nw_sb = consts.tile([128, 2], f32)
            nc.sync.dma_start(nw_sb[:], nw_d.rearrange("p2 r one -> r (p2 one)"))
            lamq_sb = consts.tile([HD, 256], f32)
            nc.sync.dma_start(lamq_sb[:], lamq_d[:])
            lamk_sb = consts.tile([HD, 2], f32)
            nc.sync.dma_start(lamk_sb[:], lamk_d[:])
            id_sb = consts.tile([128, 128], f32)
            nc.sync.dma_start(id_sb[:], id_d[:])
            el = consts.tile([128, 2], f32)
            lam_bc = consts.tile([128, 1], f32)

            # persistent qkv^T tiles: rows [hA q1|q2, hB q1|q2] per pair
            QT = [qkv.tile([128, S], bf, name=f"QT{p}") for p in range(2)]
            KT = [qkv.tile([128, S], bf, name=f"KT{p}") for p in range(2)]
            # V_aug per head: 16 chunks of [128 tok, 65] (col 64 = ones)
            VA = [vbuf.tile([128, 16 * 65], bf, name=f"VA{h}") for h in range(4)]
            for h in range(4):
                for t in range(16):
                    nc.vector.memset(VA[h][:, 65 * t + 64 : 65 * t + 65], 1.0)
            # On per head: [q, d] layout, 16 chunks of [128 tok, 64]
            ON = [onb.tile([128, 16 * HD], bf, name=f"ON{h}") for h in range(4)]

            # ---- phase 1: projections ----
            with (
                tc.tile_pool(name="pp", bufs=1, space="PSUM") as pp,
                tc.tile_pool(name="xs", bufs=3) as xs,
                tc.tile_pool(name="vt", bufs=2) as vt,
            ):
                # lambda scalar on device (replicated to all 128 partitions)
                psl = pp.tile([128, 2], f32, tag="lam")
                nc.tensor.matmul(
                    psl[:, 0:1], lamq_sb[:, 0:128], lamk_sb[:, 0:1],
                    start=True, stop=True,
                )
                nc.tensor.matmul(
                    psl[:, 1:2], lamq_sb[:, 128:256], lamk_sb[:, 1:2],
                    start=True, stop=True,
                )
                nc.scalar.activation(el[:], psl[:], FT.Exp)
                # lam_bc = (e2 - 0.8) - e1 = -(e1 - e2 + 0.8)
                nc.vector.scalar_tensor_tensor(
                    lam_bc[:], el[:, 1:2], -LAMBDA_INIT, el[:, 0:1],
                    op0=OP.add, op1=OP.subtract,
                )

                for j in range(4):  # 512-token chunks
                    accs = [
                        pp.tile([128, 512], f32, tag=f"a{m}", name=f"acc{m}")
                        for m in range(6)
                    ]
                    for k in range(8):  # e-chunks
                        xt_t = xs.tile([128, 512], bf, tag="xt")
                        nc.sync.dma_start(
                            xt_t[:],
                            xt_d[128 * k : 128 * (k + 1), 512 * j : 512 * (j + 1)],
                        )
                        for m in range(6):
                            nc.tensor.matmul(
                                accs[m][:],
                                wqv[:, m, k, :],
                                xt_t[:],
                                start=(k == 0),
                                stop=(k == 7),
                            )
                    for p in range(2):
                        nc.scalar.copy(
                            QT[p][:, 512 * j : 512 * (j + 1)], accs[0 + p][:]
                        )
                        nc.scalar.copy(
                            KT[p][:, 512 * j : 512 * (j + 1)], accs[2 + p][:]
                        )
                        vtmp = vt.tile([128, 512], f32, tag="vtmp")
                        nc.scalar.copy(vtmp[:], accs[4 + p][:])
                        for u in range(4):
                            t_idx = 4 * j + u
                            trp = pp.tile([128, 128], f32, tag="tr")
                            nc.tensor.transpose(
                                trp[:], vtmp[:, 128 * u : 128 * (u + 1)], id_sb[:]
                            )
                            for hh in range(2):
                                nc.vector.tensor_copy(
                                    VA[2 * p + hh][:, 65 * t_idx : 65 * t_idx + 64],
                                    trp[:, 64 * hh : 64 * (hh + 1)],
                                )

            # ---- phase 2: attention ----
            with (
                tc.tile_pool(name="pa", bufs=1, space="PSUM") as pa,
                tc.tile_pool(name="eb", bufs=4) as eb,
                tc.tile_pool(name="ep", bufs=2) as ep,
            ):
                def emit_qk_exp(p, jq, t):
                    es = []
                    for half in range(2):
                        qk = pa.tile([128, 1024], f32, tag=f"qk{half}", name=f"qk{half}")
                        for s in range(2):
                            u = 2 * half + s
                            nc.tensor.matmul(
                                qk[:, 512 * s : 512 * (s + 1)],
                                KT[p][32 * u : 32 * (u + 1), 128 * t : 128 * (t + 1)],
                                QT[p][32 * u : 32 * (u + 1), 512 * jq : 512 * (jq + 1)],
                                start=True,
                                stop=True,
                                tile_position=(32 * u, 0),
                            )
                        e_t = eb.tile([128, 1024], bf, tag=f"e{half}", name=f"e{half}")
                        nc.scalar.activation(e_t[:], qk[:], FT.Exp, scale=0.125)
                        es.append(e_t)
                    return es

                def emit_epilogue(p, jq, po):
                    heads = (2 * p, 2 * p + 1)
                    osb = []
                    for u in range(4):
                        o_sb = ep.tile([66, 512], f32, tag=f"osb{u}", name=f"osb{u}")
                        nc.vector.memset(o_sb[64:66, :], 0.0)
                        nc.vector.tensor_copy(o_sb[0:65, :], po[u][:])
                        osb.append(o_sb)
                    # transpose to [q, (v, 66)]
                    ot = []
                    for u in range(4):
                        trb = pa.tile([128, 4 * 66], f32, tag=f"po{u}", name=f"trb{u}")
                        for v in range(4):
                            nc.tensor.transpose(
                                trb[:, 66 * v : 66 * (v + 1)],
                                osb[u][0:66, 128 * v : 128 * (v + 1)],
                                id_sb[0:66, 0:66],
                            )
                        o_t = ep.tile([128, 4 * 66], f32, tag=f"ot{u}", name=f"ot{u}")
                        nc.vector.tensor_copy(o_t[:], trb[:])
                        ot.append(o_t)
                    # per-head math in [q, d]
                    rv = [
                        o.rearrange("p (v c) -> p v c", c=66)[:, :, 64:65] for o in ot
                    ]  # r columns [128, 4, 1]
                    rcp = ep.tile([128, 8], f32, tag="rcp")
                    nc.vector.reciprocal(
                        rcp.rearrange("p (h v) -> p h v", h=2)[:, 0, :], rv[1][:, :, 0]
                    )
                    nc.vector.reciprocal(
                        rcp.rearrange("p (h v) -> p h v", h=2)[:, 1, :], rv[3][:, :, 0]
                    )
                    nmu = ep.tile([128, 8], f32, tag="nmu")
                    for hh in range(2):
                        nc.vector.scalar_tensor_tensor(
                            nmu[:, 4 * hh : 4 * (hh + 1)],
                            rcp[:, 4 * hh : 4 * (hh + 1)],
                            lam_bc[:],
                            rv[2 * hh][:, :, 0],
                            op0=OP.mult,
                            op1=OP.mult,
                        )
                    pre = [
                        ep.tile([128, 4 * HD], f32, tag=f"pre{hh}", name=f"pre{hh}")
                        for hh in range(2)
                    ]
                    sqs = ep.tile([128, HD], f32, tag="sqs")
                    ss = ep.tile([128, 8], f32, tag="ss")
                    for hh in range(2):
                        o1 = ot[2 * hh].rearrange("p (v c) -> p v c", c=66)
                        o2 = ot[2 * hh + 1].rearrange("p (v c) -> p v c", c=66)
                        for v in range(4):
                            pv_slice = pre[hh][:, HD * v : HD * (v + 1)]
                            nc.vector.scalar_tensor_tensor(
                                pv_slice,
                                o2[:, v, 0:HD],
                                nmu[:, 4 * hh + v : 4 * hh + v + 1],
                                o1[:, v, 0:HD],
                                op0=OP.mult,
                                op1=OP.add,
                            )
                            nc.vector.tensor_tensor(
                                sqs[:], pv_slice, pv_slice, op=OP.mult
                            )
                            nc.vector.tensor_reduce(
                                ss[:, 4 * hh + v : 4 * hh + v + 1],
                                sqs[:],
                                axis=mybir.AxisListType.X,
                                op=OP.add,
                            )
                    # rsqrt(ss/64 + eps) via bit trick + 2 Newton steps
                    msc = ep.tile([128, 8], f32, tag="msc")
                    nc.vector.tensor_scalar(
                        msc[:], ss[:], 1.0 / HD, EPS, op0=OP.mult, op1=OP.add
                    )
                    y0i = ep.tile([128, 8], i32, tag="y0i")
                    nc.vector.tensor_scalar(
                        y0i[:], msc.bitcast(i32)[:], 1, None, op0=OP.arith_shift_right
                    )
                    nc.vector.tensor_scalar(
                        y0i[:], y0i[:], -1, MAGIC, op0=OP.mult, op1=OP.add
                    )
                    y = y0i.bitcast(f32)
                    t1 = ep.tile([128, 8], f32, tag="t1")
                    for _ in range(2):
                        nc.vector.tensor_tensor(t1[:], y[:], y[:], op=OP.mult)
                        nc.vector.tensor_tensor(t1[:], t1[:], msc[:], op=OP.mult)
                        nc.vector.tensor_scalar(
                            t1[:], t1[:], -0.5, 1.5, op0=OP.mult, op1=OP.add
                        )
                        nc.vector.tensor_tensor(y[:], y[:], t1[:], op=OP.mult)
                    # On = pre * rsq
                    for hh in range(2):
                        for v in range(4):
                            c_idx = 4 * jq + v
                            nc.vector.tensor_scalar(
                                ON[heads[hh]][:, HD * c_idx : HD * (c_idx + 1)],
                                pre[hh][:, HD * v : HD * (v + 1)],
                                y[:, 4 * hh + v : 4 * hh + v + 1],
                                None,
                                op0=OP.mult,
                            )

                pending_ep = None
                for p in range(2):
                    for jq in range(4):  # query chunks of 512
                        es_t = emit_qk_exp(p, jq, 0)
                        if pending_ep is not None:
                            emit_epilogue(*pending_ep)
                            pending_ep = None
                        po = [
                            pa.tile([65, 512], f32, tag=f"po{u}", name=f"po{u}")
                            for u in range(4)
                        ]
                        for t in range(16):
                            es_next = emit_qk_exp(p, jq, t + 1) if t < 15 else None
                            for u in range(4):
                                nc.tensor.matmul(
                                    po[u][:],
                                    VA[2 * p + (u >> 1)][:, 65 * t : 65 * (t + 1)],
                                    es_t[u >> 1][:, 512 * (u & 1) : 512 * ((u & 1) + 1)],
                                    start=(t == 0),
                                    stop=(t == 15),
                                )
                            es_t = es_next
                        pending_ep = (p, jq, po)
                emit_epilogue(*pending_ep)

            # ---- phase 3: wo matmul over token halves ----
            nc.sync.dma_start(
                wo_sb.rearrange("p (k j) -> p k j", k=8),
                wo_d.rearrange("(k p) j -> p k j", p=128),
            )
            with (
                tc.tile_pool(name="pw", bufs=2, space="PSUM") as pw,
                tc.tile_pool(name="fe", bufs=2) as fe,
            ):
                for p in range(2):
                    for thi in range(2):
                        for n in range(2):
                            fp = pw.tile([128, 512], f32, tag="f")
                            for k in range(8):
                                c_idx = 8 * thi + k
                                nc.tensor.matmul(
                                    fp[0:64, :],
                                    ON[2 * p][:, HD * c_idx : HD * (c_idx + 1)],
                                    wov[:, k, 512 * n : 512 * (n + 1)],
                                    start=(k == 0),
                                    stop=(k == 7),
                                    tile_position=(0, 0),
                                )
                            for k in range(8):
                                c_idx = 8 * thi + k
                                nc.tensor.matmul(
                                    fp[64:128, :],
                                    ON[2 * p + 1][:, HD * c_idx : HD * (c_idx + 1)],
                                    wov[:, k, 512 * n : 512 * (n + 1)],
                                    start=(k == 0),
                                    stop=(k == 7),
                                    tile_position=(0, 64),
                                )
                            fsb = fe.tile([128, 512], f32, tag="fsb")
                            nc.vector.tensor_scalar(
                                fsb[:], fp[:], nw_sb[:, p : p + 1], None, op0=OP.mult
                            )
                            nc.sync.dma_start(
                                out_d[2 * p, thi, :, 512 * n : 512 * (n + 1)],
                                fsb[0:64, :],
                            )
                            nc.sync.dma_start(
                                out_d[2 * p + 1, thi, :, 512 * n : 512 * (n + 1)],
                                fsb[64:128, :],
                            )

    nc.compile()
    return nc


def _shard_inputs(x, w_qkv, wo, lambda_q1, lambda_q2, lambda_k1, lambda_k2, norm_w):
    import ml_dtypes

    bf16 = ml_dtypes.bfloat16
    x = np.asarray(x, dtype=np.float32)
    w_qkv = np.asarray(w_qkv, dtype=np.float32)
    wo_b = np.asarray(wo, dtype=np.float32).astype(bf16)
    norm_w = np.asarray(norm_w, dtype=np.float32)
    lq1 = np.asarray(lambda_q1, np.float32)
    lq2 = np.asarray(lambda_q2, np.float32)
    lk1 = np.asarray(lambda_k1, np.float32)
    lk2 = np.asarray(lambda_k2, np.float32)

    lamq = np.concatenate(
        [np.repeat(lq1[:, None], 128, axis=1), np.repeat(lq2[:, None], 128, axis=1)],
        axis=1,
    ).astype(np.float32)  # [64, 256]
    lamk = np.stack([lk1, lk2], axis=1).astype(np.float32)  # [64, 2]
    ident = np.eye(128, dtype=np.float32)

    in_maps = []
    for c in range(8):
        beta, g = divmod(c, 4)
        heads = [4 * g + i for i in range(4)]
        xt = np.ascontiguousarray(x[beta].T).astype(bf16)  # [1024, 2048]
        wq = np.empty((6, E, 128), np.float32)
        for p in range(2):
            hA, hB = heads[2 * p], heads[2 * p + 1]
            for blk, base in enumerate((0, E, 2 * E)):  # q, k, v column blocks
                wq[2 * blk + p, :, 0:64] = w_qkv[:, base + 64 * hA : base + 64 * hA + 64]
                wq[2 * blk + p, :, 64:128] = w_qkv[:, base + 64 * hB : base + 64 * hB + 64]
        # device layout: wq_sb[p, (m k c)] = wq[m, 128k+p, c]
        wq = np.ascontiguousarray(
            wq.reshape(6, 8, 128, 128).transpose(2, 0, 1, 3).reshape(128, 6144)
        )
        nw = np.empty((2, 128, 1), np.float32)
        for p in range(2):
            nw[p, 0:64, 0] = norm_w[heads[2 * p]] * (1.0 - LAMBDA_INIT)
            nw[p, 64:128, 0] = norm_w[heads[2 * p + 1]] * (1.0 - LAMBDA_INIT)
        in_maps.append(
            {
                "xt": xt,
                "wq": wq.astype(bf16),
                "wo_full": wo_b,
                "nw": nw,
                "lamq": lamq,
                "lamk": lamk,
                "ident": ident,
            }
        )
    return in_maps


def kernel(**inputs):
    from concourse import bass_utils

    if "nc" not in _CACHE:
        _CACHE["nc"] = _build_nc()
    nc = _CACHE["nc"]

    in_maps = _shard_inputs(**inputs)
    res = bass_utils.run_bass_kernel_spmd(nc, in_maps, core_ids=list(range(8)))

    out = np.zeros((B, S, E), np.float32)
    for c in range(8):
        beta, g = divmod(c, 4)
        part = res.results[c]["outp"]  # [4, 2, 64, 1024] = [h_l, t_hi, d, j]
        ob = out[beta].reshape(HD, 32, E)  # s' = 32*d + 2*h + t_hi
        for hl in range(4):
            h = 4 * g + hl
            for thi in range(2):
                ob[:, 2 * h + thi, :] += part[hl, thi]
    return out
